# revision 28
# baseline (speedup 1.0000x reference)
"""Trainium2 Bass kernel for ContextQueryAttention (BiDAF-style trilinear attention).

Computes, per batch n:
    sim[c,q] = <ctx[c], wc> + <xq[q], wc> + <ctx[c] * wcq, xq[q]>
    c2q  = softmax_q(sim) @ xq                      # [C, F]
    q2c  = softmax_c(max_q sim) @ ctx               # [F]
    out  = concat([ctx, c2q, ctx*c2q, ctx*q2c], -1) # [C, 4F]

Sharding: data-parallel over batch N=64 across 8 NeuronCores (8 batches/core).

Per-core structure (per batch):
  - all PE matmuls in fp16 (fp32 runs at 4 cyc/row vs fp16 1 cyc/row; fp16's
    10 mantissa bits keep logit noise ~0.02 abs on std-32 rows so softmax
    weights are stable; l2 err ~2e-3, far under the 2e-2 gate); PSUM fp32
  - ctx cast to fp16 (ctxh) for PE use; fp32 ctx kept for term1/3/4 + stores
  - ctxT built via 32 fp16 PE transposes; the NEXT batch's casts+transposes
    are interleaved into this batch's pass 2 so pass 1 of b+1 starts ready
  - sim psum [128c, 129] per c-tile: 4 K-chunk matmuls with an augmented
    moving operand [wcq*xqT | wc] (column 128 accumulates s_ctx for free)
    plus a rank-1 (ones x s_qry) matmul for the query bias term
  - softmax over q on the free axis: DVE reduce_max(negate) -> ACT exp with
    per-partition bias, fp16 E out, fp32 accumulated row-sum
  - pass 2 per tile: E^T -> c2q matmul -> normalize + term3 -> [F:3F] store
    per tile (4KB rows), so output DMA flows through the whole batch
  - term4 (ctx * q2c broadcast) is OFF the critical path: computed during
    the NEXT batch's pass 1 (when DVE/GPS are otherwise light) into one
    [128, CT, F] buffer, then [3F:4F] half-stores fill the DMA gaps
  - ctx (term1) stored as two half DMAs per batch during the PREVIOUS
    batch's pass 1 (data prefetched 2 batches ahead), placed AFTER that
    batch's asm stores in Sync-FIFO order so a late load can never
    head-of-line-block ready store traffic
  - loads ride the ACT HWDGE ring (issued 2 batches ahead: ctx_p/xq_p
    have 4 bufs so the WAR wait on DVE lands a full batch early);
    stores the SP ring
  - last batch: term4(b-1) h1 + term1(last) are held back and issued in
    the pass1->pass2 transition window (q2c serial chain) so the store
    ring never runs dry; term4 muls+stores are paired 2/iteration
"""

import os

os.environ.setdefault("JAX_PLATFORMS", "axon")

import numpy as np

import concourse.bass as bass
import concourse.mybir as mybir
import concourse.tile as tile
from concourse import bacc, bass_isa, bass_utils
from concourse.masks import make_identity

f32 = mybir.dt.float32
f16 = mybir.dt.float16
AX = mybir.AxisListType.X
EXP = mybir.ActivationFunctionType.Exp
COPY = mybir.ActivationFunctionType.Copy
MULT = mybir.AluOpType.mult
ADD = mybir.AluOpType.add

N_CORES = 8
B = 8          # batches per core
C = 1024       # context length
Q = 128        # query length
F = 512        # feature dim
CT = C // 128  # c-tiles per batch
FC = F // 128  # f-chunks


def build_nc():
    nc = bacc.Bacc("TRN2", target_bir_lowering=False, debug=False)
    xc = nc.dram_tensor("x_context", [B, C, F], f32, kind="ExternalInput").ap()
    xq_d = nc.dram_tensor("x_query", [B, Q, F], f32, kind="ExternalInput").ap()
    wc_d = nc.dram_tensor("w_context", [F], f32, kind="ExternalInput").ap()
    wcq_d = nc.dram_tensor("w_cq", [F], f32, kind="ExternalInput").ap()
    out = nc.dram_tensor("out", [B, C, 4 * F], f32, kind="ExternalOutput").ap()

    from contextlib import ExitStack

    with tile.TileContext(nc) as tc, ExitStack() as es:
        def pool(name, bufs, space="SBUF"):
            return es.enter_context(tc.tile_pool(name=name, bufs=bufs, space=space))

        const = pool("const", 1)
        ctx_p = pool("ctx_p", 4)
        ctxh_p = pool("ctxh_p", 2)
        ctxT_p = pool("ctxT_p", 2)
        xq_p = pool("xq_p", 4)
        xqh_p = pool("xqh_p", 2)
        xqw_p = pool("xqw_p", 2)
        tmp_p = pool("tmp_p", 2)
        e_p = pool("e_p", CT + 2)
        et_p = pool("et_p", 3)
        asm_p = pool("asm_p", 6)
        t4_p = pool("t4_p", 2)
        vec_p = pool("vec_p", CT + 2)
        sml_p = pool("sml_p", 2)
        ps_sim_p = pool("ps_sim", 2, "PSUM")
        ps_ctxT_p = pool("ps_ctxT", 2, "PSUM")
        ps_c2q_p = pool("ps_c2q", 2, "PSUM")
        ps_sml_p = pool("ps_sml", 2, "PSUM")

        # loads on the ACT HWDGE ring; stores on the SP ring
        dma_load = nc.scalar.dma_start
        dma_store = nc.sync.dma_start

        def load_batch(b):
            # xq first (xq_prep is the first consumer), ctx in halves so
            # ctxT casts and term1 half-stores can start on half 0 early
            xq = xq_p.tile([128, F], f32, name="xq")
            dma_load(xq, xq_d[b])
            ctx = ctx_p.tile([128, CT, F], f32, name="ctx")
            dma_load(ctx[:, 0:4], xc[b, 0:512].rearrange("(t p) f -> p t f", p=128))
            dma_load(ctx[:, 4:CT], xc[b, 512:1024].rearrange("(t p) f -> p t f", p=128))
            return ctx, xq

        def term1_half(b, ctx, half):
            lo = half * 4
            dma_store(
                out[b, lo * 128 : (lo + 4) * 128, 0:F].rearrange(
                    "(t p) f -> p t f", p=128
                ),
                ctx[:, lo : lo + 4],
            )

        # issue the first batches' loads BEFORE the tiny const gathers -- the
        # HWDGE ring is FIFO and the 4B-per-partition wc gathers would
        # otherwise block the 2MB ctx loads for ~10us.
        bufs = {0: load_batch(0), 1: load_batch(1)}

        ident = const.tile([128, 128], f32)
        make_identity(nc, ident)
        identh = const.tile([128, 128], f16)
        nc.vector.tensor_copy(identh, ident)
        ones_rowh = const.tile([1, 128], f16)
        nc.vector.memset(ones_rowh, 1.0)
        ones_col = const.tile([128, 1], f32)
        nc.vector.memset(ones_col, 1.0)
        # const gathers go on the (idle-at-startup) store ring so their tiny
        # packets don't sit between the batch-0 and batch-1 loads
        wc_sb = const.tile([128, FC], f32)
        dma_store(wc_sb, wc_d.rearrange("(a p) -> p a", p=128))
        wc_sbh = const.tile([128, FC], f16)
        nc.vector.tensor_copy(wc_sbh, wc_sb)
        wcq_sb = const.tile([128, FC], f32)
        dma_store(wcq_sb, wcq_d.rearrange("(a p) -> p a", p=128))
        wc_row = const.tile([1, F], f32)
        dma_store(wc_row, wc_d[None, :])
        wc_rowh = const.tile([1, F], f16)
        nc.vector.tensor_copy(wc_rowh, wc_row)
        # wc broadcast along partitions (for s_qry): ones[1,128]^T @ wc[1,512]
        ps_wcb = ps_sml_p.tile([128, F], f32, tag="sml")
        nc.tensor.matmul(ps_wcb, lhsT=ones_rowh, rhs=wc_rowh, start=True, stop=True)
        wc_bc = const.tile([128, F], f32)
        nc.vector.tensor_copy(wc_bc, ps_wcb)

        # fp32 -> fp16 cast: DVE tensor_scalar (measured ~417ns/[128,512]) or
        # ACT copy (~700ns). gpsimd tensor_scalar is microcoded at ~7.7us and
        # poisons concurrent DVE access to the same partitions -- never use.
        def cast_dve(dst, src):
            nc.vector.tensor_scalar_mul(dst, src, 1.0)

        def cast_act(dst, src):
            nc.scalar.copy(dst, src)

        # ---- per-batch stage builders ----

        def xq_prep(xq):
            """xqh, xqw_aug (scaled+augmented xqT), s_qry row."""
            xqh = xqh_p.tile([128, F], f16, name="xqh")
            cast_dve(xqh, xq)
            xqw_aug = xqw_p.tile([128, FC, Q + 1], f16)
            for fc in range(FC):
                ps_xqT = ps_sml_p.tile([128, 128], f16, tag="sml")
                nc.tensor.transpose(ps_xqT, xqh[:, fc * 128 : (fc + 1) * 128], identh)
                nc.scalar.activation(
                    xqw_aug[:, fc, 0:Q], ps_xqT, COPY,
                    scale=wcq_sb[:, fc : fc + 1],
                )
                nc.vector.tensor_copy(
                    xqw_aug[:, fc, Q : Q + 1], wc_sbh[:, fc : fc + 1]
                )
            scr = tmp_p.tile([128, F], f32, name="scr", tag="scr")
            sq_col = vec_p.tile([128, 1], f32, tag="sqcol")
            nc.vector.tensor_mul(scr, xq, wc_bc)
            nc.vector.reduce_sum(sq_col, scr, axis=AX)
            ps_sqT = ps_sml_p.tile([1, 128], f32, tag="sml")
            nc.tensor.transpose(ps_sqT, sq_col, ident)
            sq_rowh = sml_p.tile([1, 128], f16, name="sq_rowh", tag="sq_row")
            nc.scalar.copy(sq_rowh, ps_sqT)
            return xqh, xqw_aug, sq_rowh

        def ctxT_casts(ctx, ctxh, half):
            for j in range(4):
                t = half * 4 + j
                (cast_dve if j % 2 == 0 else cast_act)(ctxh[:, t], ctx[:, t])

        def ctxT_fc(ctxh, ctxT, half, fc):
            ps_ct = ps_ctxT_p.tile([128, 512], f16)
            for j in range(4):
                t = half * 4 + j
                nc.tensor.transpose(
                    ps_ct[:, j * 128 : (j + 1) * 128],
                    ctxh[:, t, fc * 128 : (fc + 1) * 128],
                    identh,
                )
            cp = nc.vector.tensor_copy if fc % 2 == 0 else nc.scalar.copy
            cp(ctxT[:, fc, half * 512 : (half + 1) * 512], ps_ct)

        def ctxT_chunk(ctx, ctxh, ctxT, half):
            ctxT_casts(ctx, ctxh, half)
            for fc in range(FC):
                ctxT_fc(ctxh, ctxT, half, fc)

        def pass1_tile(t, ctxT, xqw_aug, sq_rowh, z, Es, rcps):
            ps_sim = ps_sim_p.tile([128, Q + 1], f32)
            for fc in range(FC):
                nc.tensor.matmul(
                    ps_sim,
                    lhsT=ctxT[:, fc, t * 128 : t * 128 + 128],
                    rhs=xqw_aug[:, fc],
                    start=(fc == 0),
                    stop=False,
                )
            nc.tensor.matmul(
                ps_sim[:, 0:Q], lhsT=ones_rowh, rhs=sq_rowh, start=False, stop=True
            )
            nmax = vec_p.tile([128, 1], f32, tag="nmax")
            nc.vector.reduce_max(nmax, ps_sim[:, 0:Q], axis=AX, negate=True)
            E = e_p.tile([128, Q], f16)
            rsum = vec_p.tile([128, 1], f32, tag="rsum")
            nc.scalar.activation(E, ps_sim[:, 0:Q], EXP, bias=nmax, accum_out=rsum)
            rcp = vec_p.tile([128, 1], f32, tag="rcp")
            nc.vector.reciprocal(rcp, rsum)
            # z[:, t] = s_ctx + rowmax = psum[:,128] - (-max)
            nc.vector.tensor_sub(z[:, t : t + 1], ps_sim[:, Q : Q + 1], nmax)
            Es.append(E)
            rcps.append(rcp)

        def q2c_prep(z):
            zmax = vec_p.tile([128, 1], f32, tag="zmax")
            nc.vector.reduce_max(zmax, z, axis=AX)
            gmax = vec_p.tile([128, 1], f32, tag="gmax")
            nc.gpsimd.partition_all_reduce(
                gmax, zmax, channels=128, reduce_op=bass_isa.ReduceOp.max
            )
            negb = vec_p.tile([128, 1], f32, tag="negb")
            nc.vector.tensor_scalar_mul(negb, gmax, -1.0)
            expz = sml_p.tile([128, CT], f16, name="expz", tag="expz")
            ers = vec_p.tile([128, 1], f32, tag="ers")
            nc.scalar.activation(expz, z, EXP, bias=negb, accum_out=ers)
            return expz, ers

        def q2c_matmuls(expz, ers, ctxh):
            ps_S = ps_sml_p.tile([1, 1], f32, tag="sml")
            nc.tensor.matmul(ps_S, lhsT=ers, rhs=ones_col, start=True, stop=True)
            rS = sml_p.tile([1, 1], f32, name="rS", tag="rS")
            nc.vector.reciprocal(rS, ps_S)
            ps_q2c = ps_sml_p.tile([1, F], f32, tag="sml")
            for t in range(CT):
                nc.tensor.matmul(
                    ps_q2c,
                    lhsT=expz[:, t : t + 1],
                    rhs=ctxh[:, t],
                    start=(t == 0),
                    stop=(t == CT - 1),
                )
            xq2c = sml_p.tile([1, F], f16, name="xq2c", tag="xq2c")
            nc.scalar.activation(xq2c, ps_q2c, COPY, scale=rS)
            ps_bc = ps_sml_p.tile([128, F], f32, tag="sml")
            nc.tensor.matmul(ps_bc, lhsT=ones_rowh, rhs=xq2c, start=True, stop=True)
            xq2cb = tmp_p.tile([128, F], f32, name="xq2cb", tag="xq2cb")
            nc.vector.tensor_copy(xq2cb, ps_bc)
            return xq2cb

        def stage_et(t, Es):
            ps_et = ps_sml_p.tile([128, Q], f16, tag="sml")
            nc.tensor.transpose(ps_et, Es[t], identh)
            ET = et_p.tile([128, Q], f16)
            nc.scalar.copy(ET, ps_et)
            return ET

        def stage_c2q_store(b, t, ET, ctx, xqh, rcps):
            """c2q matmul -> norm -> [F:2F] store -> term3 -> [2F:3F] store.

            Split stores: the [F:2F] piece is gated only on the norm (not
            term3), so half the tile's store data enters the ring ~0.5us
            earlier and drain granularity is finer across pacing gaps."""
            ps_c2q = ps_c2q_p.tile([128, F], f32)
            nc.tensor.matmul(ps_c2q, lhsT=ET, rhs=xqh, start=True, stop=True)
            asm = asm_p.tile([128, 2 * F], f32)
            if t % 2 == 0:
                nc.scalar.activation(asm[:, 0:F], ps_c2q, COPY, scale=rcps[t])
            else:
                nc.vector.tensor_scalar_mul(asm[:, 0:F], ps_c2q, rcps[t])
            dma_store(out[b, t * 128 : (t + 1) * 128, F : 2 * F], asm[:, 0:F])
            nc.vector.tensor_mul(asm[:, F : 2 * F], ctx[:, t], asm[:, 0:F])
            dma_store(
                out[b, t * 128 : (t + 1) * 128, 2 * F : 3 * F], asm[:, F : 2 * F]
            )

        def term4_tile(t, t4, ctx, xq2cb):
            nc.vector.tensor_mul(t4[:, t], ctx[:, t], xq2cb)

        def term4_store_qtr(b, t4, qtr):
            # quarter-stores: each gated on only 2 t4 muls, entering the
            # ring ~2 tiles earlier than the old half-stores
            lo = qtr * 2
            dma_store(
                out[b, lo * 128 : (lo + 2) * 128, 3 * F : 4 * F].rearrange(
                    "(t p) f -> p t f", p=128
                ),
                t4[:, lo : lo + 2],
            )

        # ---- main software-pipelined loop ----
        ctxh_nxt = ctxh_p.tile([128, CT, F], f16, name="ctxh")
        ctxT_nxt = ctxT_p.tile([128, FC, C], f16)
        ctxT_chunk(bufs[0][0], ctxh_nxt, ctxT_nxt, 0)
        ctxT_chunk(bufs[0][0], ctxh_nxt, ctxT_nxt, 1)
        pend = None  # (b, t4_tile, ctx, xq2cb) term4 work deferred to next pass 1

        for b in range(B):
            ctx, xq = bufs.pop(b)
            ctxh, ctxT = ctxh_nxt, ctxT_nxt
            last = b + 1 >= B
            if b + 2 < B:
                bufs[b + 2] = load_batch(b + 2)

            if b == 0:
                term1_half(0, ctx, 0)
                term1_half(0, ctx, 1)

            xqh, xqw_aug, sq_rowh = xq_prep(xq)

            # pass 1, with previous batch's term4 work interleaved and the
            # NEXT batch's term1 half-stores (data prefetched 2 batches
            # ahead, so they are ready and drain immediately).  term4 is
            # issued BEFORE the pass-1 tile so it doesn't sit behind chain
            # ops (reduce_max etc.) that wait on the PE.  For the final
            # batch, term4 h1 and term1(last) are HELD BACK to fill the
            # store-ring-dry pass1->pass2 transition window below.
            z = sml_p.tile([128, CT], f32, name="z", tag="z")
            Es = []
            rcps = []
            # t4 muls stay interleaved one-per-tile BEFORE each pass-1 tile:
            # they fill the DVE stall while it waits on the PE, delaying
            # reduce_max(t) by at most one mul.  (Two failed alternatives,
            # both reverted: gpsimd offload +44us -- DVE 2-port perf mode
            # locks gpsimd out of SBUF; dense mul block before pass 1 +23us
            # -- it delays the rmax->sub->exp spine by the whole block.)
            for t in range(CT):
                if pend is not None:
                    term4_tile(t, pend[1], pend[2], pend[3])
                pass1_tile(t, ctxT, xqw_aug, sq_rowh, z, Es, rcps)
                if t == 1 and b + 1 < B:
                    term1_half(b + 1, bufs[b + 1][0], 0)
                elif t == 5 and b + 1 < B - 1:
                    # h1 of the LAST batch's term1 is held back as filler for
                    # the final pass1->pass2 transition window
                    term1_half(b + 1, bufs[b + 1][0], 1)
                if pend is not None and t % 2 == 1:
                    qtr = (t - 1) // 2
                    if qtr < 2 or not last:
                        # the last batch's q2/q3 are held back as transition
                        # filler below
                        term4_store_qtr(pend[0], pend[1], qtr)
                        if t == CT - 1:
                            pend = None

            expz, ers = q2c_prep(z)

            if last:
                # transition filler: term1 h1 is unconditionally ready (the
                # load landed batches ago) so it goes FIRST; t4(b-1) q2/q3
                # muls trickle in behind PE-paced chain ops on the DVE queue
                # and would head-of-line-block it otherwise
                term1_half(b, ctx, 1)
                term4_store_qtr(pend[0], pend[1], 2)
                term4_store_qtr(pend[0], pend[1], 3)
                pend = None
            else:
                ctxh_nxt = ctxh_p.tile([128, CT, F], f16, name="ctxh")
                ctxT_nxt = ctxT_p.tile([128, FC, C], f16)

            # pass 2, with next batch's ctxT build interleaved.  Bulk work
            # (casts, transposes) is issued at the TOP of each iteration so
            # it fills engine idle time instead of queueing behind chain ops.
            ET0 = stage_et(0, Es)
            ET1 = stage_et(1, Es)
            stage_c2q_store(b, 0, ET0, ctx, xqh, rcps)
            stage_c2q_store(b, 1, ET1, ctx, xqh, rcps)
            if last:
                # t4 pairs below need xq2cb ASAP; for other batches the q2c
                # accum block (~5.6us of PE) is deferred to mid-pass-2 so it
                # doesn't sit between c2q matmuls and block early asm stores
                xq2cb = q2c_matmuls(expz, ers, ctxh)
            t4 = t4_p.tile([128, CT, F], f32)

            def last_t4(t):
                # last batch: no next pass 1 to hide term4 behind -- pair it
                # through this pass 2 with per-tile [3F:4F] stores, all on
                # DVE (gpsimd tensor ops incur multi-us DRAIN stalls here;
                # SWDGE cast-stores also regressed: ~60% of HWDGE rate)
                nc.vector.tensor_mul(t4[:, t], ctx[:, t], xq2cb)
                dma_store(
                    out[b, t * 128 : (t + 1) * 128, 3 * F : 4 * F], t4[:, t]
                )

            prev = None
            for t in range(2, CT):
                if not last:
                    nx, ch, cT = bufs[b + 1][0], ctxh_nxt, ctxT_nxt
                    if t == 2:
                        ctxT_casts(nx, ch, 0)
                    elif t == 3:
                        ctxT_fc(ch, cT, 0, 0)
                        ctxT_fc(ch, cT, 0, 1)
                    elif t == 4:
                        ctxT_fc(ch, cT, 0, 2)
                        ctxT_fc(ch, cT, 0, 3)
                        ctxT_casts(nx, ch, 1)
                    elif t == 5:
                        ctxT_fc(ch, cT, 1, 0)
                        ctxT_fc(ch, cT, 1, 1)
                        xq2cb = q2c_matmuls(expz, ers, ctxh)
                    elif t == 6:
                        ctxT_fc(ch, cT, 1, 2)
                        ctxT_fc(ch, cT, 1, 3)
                elif t - 2 <= 3:
                    last_t4(2 * (t - 2))
                    last_t4(2 * (t - 2) + 1)
                ET = stage_et(t, Es)
                if prev is not None:
                    stage_c2q_store(b, prev[0], prev[1], ctx, xqh, rcps)
                prev = (t, ET)
            stage_c2q_store(b, prev[0], prev[1], ctx, xqh, rcps)

            if not last:
                pend = (b, t4, ctx, xq2cb)

    nc.compile()
    return nc


_NC = None


def kernel(**inputs):
    global _NC
    if _NC is None:
        _NC = build_nc()
    xc = np.ascontiguousarray(np.asarray(inputs["x_context"], dtype=np.float32))
    xq = np.ascontiguousarray(np.asarray(inputs["x_query"], dtype=np.float32))
    wc = np.ascontiguousarray(np.asarray(inputs["w_context"], dtype=np.float32))
    wcq = np.ascontiguousarray(np.asarray(inputs["w_cq"], dtype=np.float32))
    in_maps = [
        {
            "x_context": xc[i * B : (i + 1) * B],
            "x_query": xq[i * B : (i + 1) * B],
            "w_context": wc,
            "w_cq": wcq,
        }
        for i in range(N_CORES)
    ]
    res = bass_utils.run_bass_kernel_spmd(_NC, in_maps, core_ids=list(range(N_CORES)))
    return np.concatenate([res.results[i]["out"] for i in range(N_CORES)], axis=0)

